# revision 5
# baseline (speedup 1.0000x reference)
"""Causal self-attention (B=2, T=2048, C=2048, H=16, Dh=128) on 8 TRN2 NeuronCores.

Sharding: dp=2 over batch x tp=4 over heads (4 heads/core).
  - c_attn column-parallel: each core holds W_attn columns for its 4 heads
    (q, k, v slices), computes qT/kT (head-dim major) and v directly from
    a host-pre-transposed, pre-tiled xT.
  - attention: per-head causal blocks, scoresT in (k, q) orientation; the
    softmax denominator comes free from a ones column appended to v, and
    the exp'd attT blocks feed matmul2 as stationary weights (no
    transposes in the attention inner loop).
  - c_proj row-parallel: each core computes its partial y_heads @ W_proj
    rows; the 4 partials per batch are summed on the host (unshard).

The three stages are software-pipelined per 512-row T-chunk, ordered
q,k-projections -> head-0 scores+exp -> v-projections -> remaining
heads -> deferred c_proj of the previous chunk, so ScalarE exp work
hides under TensorE matmul work of the neighboring sub-stages.

All matmuls run in bf16 (inputs pre-cast on host), accumulation fp32.
Host pre-tiles every input so all DMAs are fully contiguous.
"""

import numpy as np
import ml_dtypes

import concourse.bass as bass
import concourse.tile as tile
from concourse import bacc, mybir
from concourse.bass_utils import run_bass_kernel_spmd
from concourse.masks import make_identity, make_upper_triangular

BF16 = mybir.dt.bfloat16
F32 = mybir.dt.float32

B, T, C = 2, 2048, 2048
N_HEAD, D_HEAD = 16, 128
P = 128
KT = C // P          # 16 contraction tiles for qkv projection
NH = 4               # heads per core (tp=4)
TCH = 4              # T chunks of 512 (pipeline granularity)
VW = 129             # v width with appended ones column
WG = 4               # kt-groups per input DMA (finer grain to start PE early)
SCALE = float(1.0 / np.sqrt(D_HEAD))

_CACHE: dict = {}


def _build_program(repeat: int | None = None) -> bacc.Bacc:
    """Build the SPMD program. With `repeat`, the whole body runs inside a
    hardware For_i loop (used only for timing measurements)."""
    import contextlib
    nc = bacc.Bacc("TRN2", target_bir_lowering=False, debug=False)

    # host-pre-tiled layouts (all DMAs contiguous):
    #   xT:  (TCH, P, KT, 512)  xT[tc, p, kt, t'] = x[b][tc*512+t', kt*128+p]
    #   wq/wk/wv: (P, KT, 512)  w[p, kt, m] = W[kt*128+p, m]
    #   wp:  (P, NH, C)         wp[p, h, n] = W_proj[g*512 + h*128+p, n]
    xT_h = nc.dram_tensor("xT", (TCH, P, KT, 512), BF16, kind="ExternalInput")
    wq_h = nc.dram_tensor("wq", (P, KT, NH * P), BF16, kind="ExternalInput")
    wk_h = nc.dram_tensor("wk", (P, KT, NH * P), BF16, kind="ExternalInput")
    wv_h = nc.dram_tensor("wv", (P, KT, NH * P), BF16, kind="ExternalInput")
    wp_h = nc.dram_tensor("wp", (P, NH, C), BF16, kind="ExternalInput")
    out_h = nc.dram_tensor("out", (T, C), BF16, kind="ExternalOutput")

    xT_d = xT_h.ap()
    wq_d, wk_d, wv_d, wp_d = wq_h.ap(), wk_h.ap(), wv_h.ap(), wp_h.ap()
    out_d = out_h.ap().rearrange("(mt p) n -> p mt n", p=P)    # (128, 16, 2048)

    with tile.TileContext(nc) as tc_:
        with (
            tc_.tile_pool(name="consts", bufs=1) as consts,
            tc_.tile_pool(name="persist", bufs=1) as persist,
            tc_.tile_pool(name="wpool", bufs=1) as wpool,
            tc_.tile_pool(name="xpool", bufs=2) as xpool,
            tc_.tile_pool(name="attp", bufs=21) as attp,
            tc_.tile_pool(name="ytp", bufs=2) as ytp,
            tc_.tile_pool(name="ynorm", bufs=3) as ynorm,
            tc_.tile_pool(name="osb", bufs=2) as osb,
            tc_.tile_pool(name="psA", bufs=2, space="PSUM") as psA,
            tc_.tile_pool(name="psS", bufs=2, space="PSUM") as psS,
            tc_.tile_pool(name="psY", bufs=1, space="PSUM") as psY,
            tc_.tile_pool(name="psT", bufs=1, space="PSUM") as psT,
            tc_.tile_pool(name="psO", bufs=2, space="PSUM") as psO,
        ):
            ident = consts.tile([P, P], BF16)
            tri = consts.tile([P, P], BF16)
            make_identity(nc, ident)
            # tri[i, j] = 1 where i <= j (keep k <= q), else 0
            make_upper_triangular(nc, tri, val=1.0, diag=True)

            qT = [persist.tile([P, T], BF16, tag=f"qT{h}", name=f"qT{h}")
                  for h in range(NH)]
            kTt = [persist.tile([P, T], BF16, tag=f"kT{h}", name=f"kT{h}")
                   for h in range(NH)]
            # v_aug for all heads: va_all[:, kk, h, 0:128]=v, [...,128]=1.0
            va_all = persist.tile([P, KT, NH, VW], BF16, tag="va")
            wp_sb = persist.tile([P, NH, C], BF16, tag="wp")

            wq_sb = wpool.tile([P, KT, NH * P], BF16, tag="wq")
            wk_sb = wpool.tile([P, KT, NH * P], BF16, tag="wk")
            wv_sb = wpool.tile([P, KT, NH * P], BF16, tag="wv")

            # first x chunk + k weights first, in the exact order the PE
            # consumes them (gen_qk does the k-projection before q), at
            # 2-kt granularity so the first matmul starts ~2.5us in
            xc0 = xpool.tile([P, KT, 512], BF16, tag="xc", name="xc0")
            FG = 8                      # fine groups for the critical path
            KGF = KT // FG
            for g in range(FG):
                gs = slice(g * KGF, (g + 1) * KGF)
                nc.sync.dma_start(xc0[:, gs], xT_d[0, :, gs])
                nc.sync.dma_start(wk_sb[:, gs], wk_d[:, gs])
            KG = KT // WG
            for g in range(WG):
                gs = slice(g * KG, (g + 1) * KG)
                nc.sync.dma_start(wq_sb[:, gs], wq_d[:, gs])
            for g in range(WG):
                gs = slice(g * KG, (g + 1) * KG)
                nc.sync.dma_start(wv_sb[:, gs], wv_d[:, gs])
            nc.sync.dma_start(wp_sb[:], wp_d[:])
            # ones column for free softmax denominators
            nc.vector.memset(va_all[:, :, :, D_HEAD:VW], 1.0)

            loop_ctx = tc_.For_i(0, repeat, 1) if repeat else contextlib.nullcontext()

            def emit_proj(tcu, ytc):
                # ---- Stage C(tc): c_proj partial for rows of chunk tc ----
                # emitted after B(tc+1) so it backfills PE during exp stalls
                for j in range(4):
                    mt = tcu * 4 + j
                    o = osb.tile([P, C], BF16, tag="o", name=f"o_{mt}")
                    for nck in range(4):
                        ns = slice(nck * 512, (nck + 1) * 512)
                        po = psO.tile([P, 512], F32, tag="psO", name=f"psO_{mt}_{nck}")
                        for h in range(NH):
                            nc.tensor.matmul(
                                po, ytc[h][:, j * P:(j + 1) * P], wp_sb[:, h, ns],
                                start=(h == 0), stop=(h == NH - 1),
                            )
                        nc.vector.tensor_copy(o[:, ns], po)
                        nc.sync.dma_start(out_d[:, mt, ns], o[:, ns])

            with loop_ctx:
                prev_ytc = None
                xcs = xc0
                for tcu in range(TCH):
                    ts = slice(tcu * 512, (tcu + 1) * 512)
                    xc = xcs
                    if tcu + 1 < TCH:  # prefetch next chunk
                        xcs = xpool.tile([P, KT, 512], BF16, tag="xc",
                                         name=f"xc{tcu + 1}")
                        nc.sync.dma_start(xcs[:], xT_d[tcu + 1])

                    def gen_qk(h2):
                        # q+k projection for head h2, yielding per matmul so
                        # heads 1-3 can fill att@v of the preceding head (the
                        # consumer, attn_scores(h2), runs after mm2(h2-1))
                        hs = slice(h2 * P, (h2 + 1) * P)
                        pk = psA.tile([P, 512], F32, tag="psA",
                                      name=f"pk_{tcu}_{h2}")
                        for kk in range(KT):
                            nc.tensor.matmul(
                                pk, wk_sb[:, kk, hs], xc[:, kk, :],
                                start=(kk == 0), stop=(kk == KT - 1),
                            )
                            yield True
                        nc.vector.tensor_copy(kTt[h2][:, ts], pk)
                        pq = psA.tile([P, 512], F32, tag="psA",
                                      name=f"pq_{tcu}_{h2}")
                        for kk in range(KT):
                            nc.tensor.matmul(
                                pq, wq_sb[:, kk, hs], xc[:, kk, :],
                                start=(kk == 0), stop=(kk == KT - 1),
                            )
                            yield True
                        nc.vector.tensor_copy(qT[h2][:, ts], pq)

                    # ---- Stage A(tc): only head 0's q,k up front ----
                    for _ in gen_qk(0):
                        pass

                    qc = tcu

                    def attn_scores(h):
                        # mm1 + exp (+ causal mask) for one head's q chunk
                        att = []
                        for kk in range(4 * qc + 4):
                            ps = psS.tile([P, 512], F32, tag="psS")
                            a = attp.tile([P, 512], BF16, tag="att")
                            if kk < 4 * qc:
                                nc.tensor.matmul(
                                    ps, kTt[h][:, kk * P:(kk + 1) * P], qT[h][:, ts],
                                    start=True, stop=True,
                                )
                                nc.scalar.activation(
                                    a[:], ps[:],
                                    mybir.ActivationFunctionType.Exp, scale=SCALE,
                                )
                            else:
                                off = (kk - 4 * qc) * P
                                nc.tensor.matmul(
                                    ps[:, off:512],
                                    kTt[h][:, kk * P:(kk + 1) * P],
                                    qT[h][:, qc * 512 + off:(qc + 1) * 512],
                                    start=True, stop=True,
                                )
                                nc.scalar.activation(
                                    a[:, off:512], ps[:, off:512],
                                    mybir.ActivationFunctionType.Exp, scale=SCALE,
                                )
                                nc.vector.tensor_tensor(
                                    a[:, off:off + P], a[:, off:off + P], tri,
                                    mybir.AluOpType.mult,
                                )
                            att.append(a)
                        return att

                    # head 0 scores/exp pulled forward: ScalarE works on them
                    # while PE does the v-projections below
                    att0 = attn_scores(0)

                    def gen_vproj(tt):
                        # one v-projection column tile; yields per matmul so it
                        # can be woven between att@v matmuls as 512-col filler
                        pv = psA.tile([P, 512], F32, tag="psA",
                                      name=f"pv_{tcu}_{tt}")
                        for kk in range(KT):
                            nc.tensor.matmul(
                                pv, xc[:, kk, tt * P:(tt + 1) * P],
                                wv_sb[:, kk, :],
                                start=(kk == 0), stop=(kk == KT - 1),
                            )
                            yield True
                        nc.vector.tensor_copy(
                            va_all[:, tcu * 4 + tt, :, 0:D_HEAD],
                            pv.rearrange("p (h d) -> p h d", h=NH),
                        )

                    def gen_cproj(tcv, ytcv):
                        # c_proj of the previous chunk, yielding per matmul
                        for jmt in range(4):
                            mt = tcv * 4 + jmt
                            o = osb.tile([P, C], BF16, tag="o", name=f"o_{mt}")
                            for nck in range(4):
                                ns = slice(nck * 512, (nck + 1) * 512)
                                po = psO.tile([P, 512], F32, tag="psO",
                                              name=f"psO_{mt}_{nck}")
                                for h2 in range(NH):
                                    nc.tensor.matmul(
                                        po, ytcv[h2][:, jmt * P:(jmt + 1) * P],
                                        wp_sb[:, h2, ns],
                                        start=(h2 == 0), stop=(h2 == NH - 1),
                                    )
                                    yield True
                                nc.vector.tensor_copy(o[:, ns], po)
                                nc.sync.dma_start(out_d[:, mt, ns], o[:, ns])

                    # v tiles whose diag-block consumers come too early to be
                    # fed by fillers are emitted up front; the rest (plus the
                    # deferred c_proj) interleave 1:1 with the narrow att@v
                    # matmuls, hiding their ~139ns issue floor under 512-col
                    # streams (measured: a 512+129 pair costs ~305ns vs 368).
                    import itertools
                    if tcu == 0:
                        up_tt = 4
                    elif tcu == 1:
                        up_tt = 3
                    else:
                        up_tt = 2
                    for tt in range(up_tt):
                        for _ in gen_vproj(tt):
                            pass
                    # per-head primary fillers: mm2(h) hides vproj tails /
                    # the NEXT head's q,k projection; shared tail = c_proj
                    primary = {
                        0: itertools.chain(
                            *[gen_vproj(tt) for tt in range(up_tt, 4)],
                            gen_qk(1)),
                        1: gen_qk(2),
                        2: gen_qk(3),
                        3: iter(()),
                    }
                    tail = (gen_cproj(tcu - 1, prev_ytc)
                            if prev_ytc is not None else iter(()))

                    # ---- Stage B(qc=tc): remaining heads + weighted sums ----
                    ytc = []
                    for h in range(NH):
                        att = att0 if h == 0 else attn_scores(h)

                        yt = ytp.tile([P, 512], BF16, tag=f"yT{h}", name=f"yT{h}_{qc}")
                        ytc.append(yt)
                        pri = primary[h]
                        for j in range(4):
                            qt = qc * 4 + j
                            py = psY.tile([P, VW], F32, tag="psY")
                            for kk in range(qt + 1):
                                nc.tensor.matmul(
                                    py, att[kk][:, j * P:(j + 1) * P],
                                    va_all[:, kk, h, :],
                                    start=(kk == 0), stop=(kk == qt),
                                )
                                if next(pri, None) is None:
                                    next(tail, None)
                            r = ynorm.tile([P, 1], F32, tag="r")
                            nc.vector.reciprocal(r, py[:, D_HEAD:VW])
                            y = ynorm.tile([P, P], BF16, tag="y")
                            nc.vector.tensor_scalar_mul(y, py[:, 0:D_HEAD], r)
                            pt = psT.tile([P, P], BF16, tag="psT")
                            nc.tensor.transpose(pt, y, ident)
                            nc.vector.tensor_copy(yt[:, j * P:(j + 1) * P], pt)
                        # next head's q,k must be complete before its scores
                        for _ in pri:
                            pass

                    for _ in tail:
                        pass
                    prev_ytc = ytc

                emit_proj(TCH - 1, prev_ytc)

    nc.compile()
    return nc


def _get_program() -> bacc.Bacc:
    if "nc" not in _CACHE:
        _CACHE["nc"] = _build_program()
    return _CACHE["nc"]


def _make_in_maps(x, W_attn, W_proj):
    bf = ml_dtypes.bfloat16
    x = np.asarray(x, dtype=np.float32)
    W_attn = np.asarray(W_attn, dtype=np.float32)
    W_proj = np.asarray(W_proj, dtype=np.float32)

    # xT[tc, p, kt, t'] = x[b][tc*512+t', kt*128+p]
    xT_b = []
    for b in range(B):
        xt = x[b].T.reshape(KT, P, TCH, 512).transpose(2, 1, 0, 3)
        xT_b.append(np.ascontiguousarray(xt).astype(bf))

    def _tile_w(w):  # (C, 512) -> (P, KT, 512)
        return np.ascontiguousarray(
            w.reshape(KT, P, NH * P).transpose(1, 0, 2)).astype(bf)

    GW = NH * D_HEAD  # 512 columns per tp group
    in_maps = []
    for core in range(8):
        b, g = divmod(core, 4)
        wp = W_proj[g * GW:(g + 1) * GW, :].reshape(NH, P, C).transpose(1, 0, 2)
        in_maps.append({
            "xT": xT_b[b],
            "wq": _tile_w(W_attn[:, g * GW:(g + 1) * GW]),
            "wk": _tile_w(W_attn[:, C + g * GW: C + (g + 1) * GW]),
            "wv": _tile_w(W_attn[:, 2 * C + g * GW: 2 * C + (g + 1) * GW]),
            "wp": np.ascontiguousarray(wp).astype(bf),
        })
    return in_maps


def kernel(x, W_attn, W_proj, _want_results=False, _trace=False):
    nc = _get_program()
    in_maps = _make_in_maps(x, W_attn, W_proj)
    res = run_bass_kernel_spmd(
        nc, in_maps, core_ids=list(range(8)), trace=_trace,
    )
    parts = [np.asarray(res.results[i]["out"], dtype=np.float32) for i in range(8)]
    out = np.stack([
        parts[0] + parts[1] + parts[2] + parts[3],
        parts[4] + parts[5] + parts[6] + parts[7],
    ]).astype(np.float32)
    if _want_results:
        return out, res
    return out



# revision 6
# speedup vs baseline: 1.2486x; 1.2486x over previous
"""Causal self-attention (B=2, T=2048, C=2048, H=16, Dh=128) on 8 TRN2 NeuronCores.

Sharding: dp=2 over batch x tp=4 over heads (4 heads/core).
  - c_attn column-parallel: each core holds W_attn columns for its 4 heads
    (q, k, v slices), computes qT/kT (head-dim major) and v directly from
    a host-pre-transposed, pre-tiled xT.
  - attention: per-head causal blocks, scoresT in (k, q) orientation; the
    softmax denominator comes free from a ones column appended to v, and
    the exp'd attT blocks feed matmul2 as stationary weights (no
    transposes in the attention inner loop).
  - c_proj row-parallel: each core computes its partial y_heads @ W_proj
    rows; the 4 partials per batch are summed on the host (unshard).

The three stages are software-pipelined per 512-row T-chunk, ordered
q,k-projections -> head-0 scores+exp -> v-projections -> remaining
heads -> deferred c_proj of the previous chunk, so ScalarE exp work
hides under TensorE matmul work of the neighboring sub-stages.

Cold-start path: startup DMAs are issued in exact PE consumption order
(x chunk + k weights interleaved at 2-kt grain, then q, v, proj
weights), and chunk 0 interleaves all four heads' k-projections at the
same 2-kt grain so the PE drains each arriving DMA group slower than
the next one lands (borrowing 2 idle c_proj PSUM banks). Partial
outputs are written in bf16 (the host sums them in f32), with one DMA
per 512-column PSUM group so the drain tail stays short.

All matmuls run in bf16 (inputs pre-cast on host), accumulation fp32.
Host pre-tiles every input so all DMAs are fully contiguous.
"""

import numpy as np
import ml_dtypes

import concourse.bass as bass
import concourse.tile as tile
from concourse import bacc, mybir
from concourse.bass_utils import run_bass_kernel_spmd
from concourse.masks import make_identity, make_upper_triangular

BF16 = mybir.dt.bfloat16
F32 = mybir.dt.float32

B, T, C = 2, 2048, 2048
N_HEAD, D_HEAD = 16, 128
P = 128
KT = C // P          # 16 contraction tiles for qkv projection
NH = 4               # heads per core (tp=4)
TCH = 4              # T chunks of 512 (pipeline granularity)
VW = 129             # v width with appended ones column
WG = 4               # kt-groups per input DMA (finer grain to start PE early)
SCALE = float(1.0 / np.sqrt(D_HEAD))

_CACHE: dict = {}


def _build_program(repeat: int | None = None) -> bacc.Bacc:
    """Build the SPMD program. With `repeat`, the whole body runs inside a
    hardware For_i loop (used only for timing measurements)."""
    import contextlib
    nc = bacc.Bacc("TRN2", target_bir_lowering=False, debug=False)

    # host-pre-tiled layouts (all DMAs contiguous):
    #   xT:  (TCH, P, KT, 512)  xT[tc, p, kt, t'] = x[b][tc*512+t', kt*128+p]
    #   wq/wk/wv: (P, KT, 512)  w[p, kt, m] = W[kt*128+p, m]
    #   wp:  (P, NH, C)         wp[p, h, n] = W_proj[g*512 + h*128+p, n]
    xT_h = nc.dram_tensor("xT", (TCH, P, KT, 512), BF16, kind="ExternalInput")
    wq_h = nc.dram_tensor("wq", (P, KT, NH * P), BF16, kind="ExternalInput")
    wk_h = nc.dram_tensor("wk", (P, KT, NH * P), BF16, kind="ExternalInput")
    wv_h = nc.dram_tensor("wv", (P, KT, NH * P), BF16, kind="ExternalInput")
    wp_h = nc.dram_tensor("wp", (P, NH, C), BF16, kind="ExternalInput")
    out_h = nc.dram_tensor("out", (T, C), BF16, kind="ExternalOutput")

    xT_d = xT_h.ap()
    wq_d, wk_d, wv_d, wp_d = wq_h.ap(), wk_h.ap(), wv_h.ap(), wp_h.ap()
    out_d = out_h.ap().rearrange("(mt p) n -> p mt n", p=P)    # (128, 16, 2048)

    with tile.TileContext(nc) as tc_:
        with (
            tc_.tile_pool(name="consts", bufs=1) as consts,
            tc_.tile_pool(name="persist", bufs=1) as persist,
            tc_.tile_pool(name="wpool", bufs=1) as wpool,
            tc_.tile_pool(name="xpool", bufs=2) as xpool,
            tc_.tile_pool(name="attp", bufs=21) as attp,
            tc_.tile_pool(name="ytp", bufs=2) as ytp,
            tc_.tile_pool(name="ynorm", bufs=3) as ynorm,
            tc_.tile_pool(name="osb", bufs=2) as osb,
            tc_.tile_pool(name="psA", bufs=2, space="PSUM") as psA,
            tc_.tile_pool(name="psS", bufs=2, space="PSUM") as psS,
            tc_.tile_pool(name="psY", bufs=1, space="PSUM") as psY,
            tc_.tile_pool(name="psT", bufs=1, space="PSUM") as psT,
            tc_.tile_pool(name="psO", bufs=2, space="PSUM") as psO,
        ):
            ident = consts.tile([P, P], BF16)
            tri = consts.tile([P, P], BF16)
            make_identity(nc, ident)
            # tri[i, j] = 1 where i <= j (keep k <= q), else 0
            make_upper_triangular(nc, tri, val=1.0, diag=True)

            qT = [persist.tile([P, T], BF16, tag=f"qT{h}", name=f"qT{h}")
                  for h in range(NH)]
            kTt = [persist.tile([P, T], BF16, tag=f"kT{h}", name=f"kT{h}")
                   for h in range(NH)]
            # v_aug for all heads: va_all[:, kk, h, 0:128]=v, [...,128]=1.0
            va_all = persist.tile([P, KT, NH, VW], BF16, tag="va")
            wp_sb = persist.tile([P, NH, C], BF16, tag="wp")

            wq_sb = wpool.tile([P, KT, NH * P], BF16, tag="wq")
            wk_sb = wpool.tile([P, KT, NH * P], BF16, tag="wk")
            wv_sb = wpool.tile([P, KT, NH * P], BF16, tag="wv")

            # first x chunk + k weights first, in the exact order the PE
            # consumes them (gen_qk does the k-projection before q), at
            # 2-kt granularity so the first matmul starts ~2.5us in
            xc0 = xpool.tile([P, KT, 512], BF16, tag="xc", name="xc0")
            FG = 8                      # fine groups for the critical path
            KGF = KT // FG
            for g in range(FG):
                gs = slice(g * KGF, (g + 1) * KGF)
                nc.sync.dma_start(xc0[:, gs], xT_d[0, :, gs])
                nc.sync.dma_start(wk_sb[:, gs], wk_d[:, gs])
            KG = KT // WG
            for g in range(WG):
                gs = slice(g * KG, (g + 1) * KG)
                nc.sync.dma_start(wq_sb[:, gs], wq_d[:, gs])
            for g in range(WG):
                gs = slice(g * KG, (g + 1) * KG)
                nc.sync.dma_start(wv_sb[:, gs], wv_d[:, gs])
            nc.sync.dma_start(wp_sb[:], wp_d[:])
            # ones column for free softmax denominators
            nc.vector.memset(va_all[:, :, :, D_HEAD:VW], 1.0)

            loop_ctx = tc_.For_i(0, repeat, 1) if repeat else contextlib.nullcontext()

            def emit_proj(tcu, ytc):
                # ---- Stage C(tc): c_proj partial for rows of chunk tc ----
                # emitted after B(tc+1) so it backfills PE during exp stalls
                for j in range(4):
                    mt = tcu * 4 + j
                    o = osb.tile([P, C], BF16, tag="o", name=f"o_{mt}")
                    for nck in range(4):
                        ns = slice(nck * 512, (nck + 1) * 512)
                        po = psO.tile([P, 512], F32, tag="psO", name=f"psO_{mt}_{nck}")
                        for h in range(NH):
                            nc.tensor.matmul(
                                po, ytc[h][:, j * P:(j + 1) * P], wp_sb[:, h, ns],
                                start=(h == 0), stop=(h == NH - 1),
                            )
                        nc.vector.tensor_copy(o[:, ns], po)
                        nc.sync.dma_start(out_d[:, mt, ns], o[:, ns])

            with loop_ctx:
                prev_ytc = None
                xcs = xc0
                for tcu in range(TCH):
                    ts = slice(tcu * 512, (tcu + 1) * 512)
                    xc = xcs
                    if tcu + 1 < TCH:  # prefetch next chunk
                        xcs = xpool.tile([P, KT, 512], BF16, tag="xc",
                                         name=f"xc{tcu + 1}")
                        nc.sync.dma_start(xcs[:], xT_d[tcu + 1])

                    def gen_qk(h2):
                        # q+k projection for head h2, yielding per matmul so
                        # heads 1-3 can fill att@v of the preceding head (the
                        # consumer, attn_scores(h2), runs after mm2(h2-1))
                        hs = slice(h2 * P, (h2 + 1) * P)
                        pk = psA.tile([P, 512], F32, tag="psA",
                                      name=f"pk_{tcu}_{h2}")
                        for kk in range(KT):
                            nc.tensor.matmul(
                                pk, wk_sb[:, kk, hs], xc[:, kk, :],
                                start=(kk == 0), stop=(kk == KT - 1),
                            )
                            yield True
                        nc.vector.tensor_copy(kTt[h2][:, ts], pk)
                        pq = psA.tile([P, 512], F32, tag="psA",
                                      name=f"pq_{tcu}_{h2}")
                        for kk in range(KT):
                            nc.tensor.matmul(
                                pq, wq_sb[:, kk, hs], xc[:, kk, :],
                                start=(kk == 0), stop=(kk == KT - 1),
                            )
                            yield True
                        nc.vector.tensor_copy(qT[h2][:, ts], pq)

                    # ---- Stage A(tc) ----
                    if tcu == 0:
                        # cold start: DMA delivers (xc,wk) 2-kt groups every
                        # ~1.5us; interleave all 4 heads' k-projections at
                        # that grain so the PE consumes each group slower
                        # than it arrives (4 heads x 2 kt x 512 cols =
                        # 1.7us/group), instead of stalling on one head.
                        # psO is free until the first c_proj (chunk 1), so
                        # borrow 2 of its banks for heads 2-3.
                        kps = [psA.tile([P, 512], F32, tag="psA",
                                        name=f"pk0_{h}") for h in (0, 1)]
                        kps += [psO.tile([P, 512], F32, tag="psO",
                                         name=f"pk0_{h}") for h in (2, 3)]
                        for g2 in range(KT // 2):
                            for h in range(NH):
                                for kk in (2 * g2, 2 * g2 + 1):
                                    nc.tensor.matmul(
                                        kps[h], wk_sb[:, kk, h * P:(h + 1) * P],
                                        xc[:, kk, :],
                                        start=(kk == 0), stop=(kk == KT - 1),
                                    )
                        for h in range(NH):
                            nc.vector.tensor_copy(kTt[h][:, ts], kps[h])
                        # q-projection of head 0 only; heads 1-3 stay in the
                        # stage-B weave as mm2 fillers
                        pq0 = psA.tile([P, 512], F32, tag="psA", name="pq0_0")
                        for kk in range(KT):
                            nc.tensor.matmul(
                                pq0, wq_sb[:, kk, 0:P], xc[:, kk, :],
                                start=(kk == 0), stop=(kk == KT - 1),
                            )
                        nc.vector.tensor_copy(qT[0][:, ts], pq0)

                        def gen_q(h2):
                            hs = slice(h2 * P, (h2 + 1) * P)
                            pq = psA.tile([P, 512], F32, tag="psA",
                                          name=f"pq0_{h2}")
                            for kk in range(KT):
                                nc.tensor.matmul(
                                    pq, wq_sb[:, kk, hs], xc[:, kk, :],
                                    start=(kk == 0), stop=(kk == KT - 1),
                                )
                                yield True
                            nc.vector.tensor_copy(qT[h2][:, ts], pq)
                    else:
                        for _ in gen_qk(0):
                            pass

                    qc = tcu

                    def attn_scores(h):
                        # mm1 + exp (+ causal mask) for one head's q chunk
                        att = []
                        for kk in range(4 * qc + 4):
                            ps = psS.tile([P, 512], F32, tag="psS")
                            a = attp.tile([P, 512], BF16, tag="att")
                            if kk < 4 * qc:
                                nc.tensor.matmul(
                                    ps, kTt[h][:, kk * P:(kk + 1) * P], qT[h][:, ts],
                                    start=True, stop=True,
                                )
                                nc.scalar.activation(
                                    a[:], ps[:],
                                    mybir.ActivationFunctionType.Exp, scale=SCALE,
                                )
                            else:
                                off = (kk - 4 * qc) * P
                                nc.tensor.matmul(
                                    ps[:, off:512],
                                    kTt[h][:, kk * P:(kk + 1) * P],
                                    qT[h][:, qc * 512 + off:(qc + 1) * 512],
                                    start=True, stop=True,
                                )
                                nc.scalar.activation(
                                    a[:, off:512], ps[:, off:512],
                                    mybir.ActivationFunctionType.Exp, scale=SCALE,
                                )
                                nc.vector.tensor_tensor(
                                    a[:, off:off + P], a[:, off:off + P], tri,
                                    mybir.AluOpType.mult,
                                )
                            att.append(a)
                        return att

                    # head 0 scores/exp pulled forward: ScalarE works on them
                    # while PE does the v-projections below
                    att0 = attn_scores(0)

                    def gen_vproj(tt):
                        # one v-projection column tile; yields per matmul so it
                        # can be woven between att@v matmuls as 512-col filler
                        pv = psA.tile([P, 512], F32, tag="psA",
                                      name=f"pv_{tcu}_{tt}")
                        for kk in range(KT):
                            nc.tensor.matmul(
                                pv, xc[:, kk, tt * P:(tt + 1) * P],
                                wv_sb[:, kk, :],
                                start=(kk == 0), stop=(kk == KT - 1),
                            )
                            yield True
                        nc.vector.tensor_copy(
                            va_all[:, tcu * 4 + tt, :, 0:D_HEAD],
                            pv.rearrange("p (h d) -> p h d", h=NH),
                        )

                    def gen_cproj(tcv, ytcv):
                        # c_proj of the previous chunk, yielding per matmul
                        for jmt in range(4):
                            mt = tcv * 4 + jmt
                            o = osb.tile([P, C], BF16, tag="o", name=f"o_{mt}")
                            for nck in range(4):
                                ns = slice(nck * 512, (nck + 1) * 512)
                                po = psO.tile([P, 512], F32, tag="psO",
                                              name=f"psO_{mt}_{nck}")
                                for h2 in range(NH):
                                    nc.tensor.matmul(
                                        po, ytcv[h2][:, jmt * P:(jmt + 1) * P],
                                        wp_sb[:, h2, ns],
                                        start=(h2 == 0), stop=(h2 == NH - 1),
                                    )
                                    yield True
                                nc.vector.tensor_copy(o[:, ns], po)
                                nc.sync.dma_start(out_d[:, mt, ns], o[:, ns])

                    # v tiles whose diag-block consumers come too early to be
                    # fed by fillers are emitted up front; the rest (plus the
                    # deferred c_proj) interleave 1:1 with the narrow att@v
                    # matmuls, hiding their ~139ns issue floor under 512-col
                    # streams (measured: a 512+129 pair costs ~305ns vs 368).
                    import itertools
                    if tcu == 0:
                        up_tt = 4
                    elif tcu == 1:
                        up_tt = 3
                    else:
                        up_tt = 2
                    for tt in range(up_tt):
                        for _ in gen_vproj(tt):
                            pass
                    # per-head primary fillers: mm2(h) hides vproj tails /
                    # the NEXT head's q,k projection; shared tail = c_proj
                    if tcu == 0:
                        primary = {
                            0: gen_q(1),
                            1: gen_q(2),
                            2: gen_q(3),
                            3: iter(()),
                        }
                    else:
                        primary = {
                            0: itertools.chain(
                                *[gen_vproj(tt) for tt in range(up_tt, 4)],
                                gen_qk(1)),
                            1: gen_qk(2),
                            2: gen_qk(3),
                            3: iter(()),
                        }
                    tail = (gen_cproj(tcu - 1, prev_ytc)
                            if prev_ytc is not None else iter(()))

                    # ---- Stage B(qc=tc): remaining heads + weighted sums ----
                    ytc = []
                    for h in range(NH):
                        att = att0 if h == 0 else attn_scores(h)

                        yt = ytp.tile([P, 512], BF16, tag=f"yT{h}", name=f"yT{h}_{qc}")
                        ytc.append(yt)
                        pri = primary[h]
                        for j in range(4):
                            qt = qc * 4 + j
                            py = psY.tile([P, VW], F32, tag="psY")
                            for kk in range(qt + 1):
                                nc.tensor.matmul(
                                    py, att[kk][:, j * P:(j + 1) * P],
                                    va_all[:, kk, h, :],
                                    start=(kk == 0), stop=(kk == qt),
                                )
                                if next(pri, None) is None:
                                    next(tail, None)
                            r = ynorm.tile([P, 1], F32, tag="r")
                            nc.vector.reciprocal(r, py[:, D_HEAD:VW])
                            y = ynorm.tile([P, P], BF16, tag="y")
                            nc.vector.tensor_scalar_mul(y, py[:, 0:D_HEAD], r)
                            pt = psT.tile([P, P], BF16, tag="psT")
                            nc.tensor.transpose(pt, y, ident)
                            nc.vector.tensor_copy(yt[:, j * P:(j + 1) * P], pt)
                        # next head's q,k must be complete before its scores
                        for _ in pri:
                            pass

                    for _ in tail:
                        pass
                    prev_ytc = ytc

                emit_proj(TCH - 1, prev_ytc)

    nc.compile()
    return nc


def _get_program() -> bacc.Bacc:
    if "nc" not in _CACHE:
        _CACHE["nc"] = _build_program()
    return _CACHE["nc"]


def _make_in_maps(x, W_attn, W_proj):
    bf = ml_dtypes.bfloat16
    x = np.asarray(x, dtype=np.float32)
    W_attn = np.asarray(W_attn, dtype=np.float32)
    W_proj = np.asarray(W_proj, dtype=np.float32)

    # xT[tc, p, kt, t'] = x[b][tc*512+t', kt*128+p]
    xT_b = []
    for b in range(B):
        xt = x[b].T.reshape(KT, P, TCH, 512).transpose(2, 1, 0, 3)
        xT_b.append(np.ascontiguousarray(xt).astype(bf))

    def _tile_w(w):  # (C, 512) -> (P, KT, 512)
        return np.ascontiguousarray(
            w.reshape(KT, P, NH * P).transpose(1, 0, 2)).astype(bf)

    GW = NH * D_HEAD  # 512 columns per tp group
    in_maps = []
    for core in range(8):
        b, g = divmod(core, 4)
        wp = W_proj[g * GW:(g + 1) * GW, :].reshape(NH, P, C).transpose(1, 0, 2)
        in_maps.append({
            "xT": xT_b[b],
            "wq": _tile_w(W_attn[:, g * GW:(g + 1) * GW]),
            "wk": _tile_w(W_attn[:, C + g * GW: C + (g + 1) * GW]),
            "wv": _tile_w(W_attn[:, 2 * C + g * GW: 2 * C + (g + 1) * GW]),
            "wp": np.ascontiguousarray(wp).astype(bf),
        })
    return in_maps


def kernel(x, W_attn, W_proj, _want_results=False, _trace=False):
    nc = _get_program()
    in_maps = _make_in_maps(x, W_attn, W_proj)
    res = run_bass_kernel_spmd(
        nc, in_maps, core_ids=list(range(8)), trace=_trace,
    )
    parts = [np.asarray(res.results[i]["out"], dtype=np.float32) for i in range(8)]
    out = np.stack([
        parts[0] + parts[1] + parts[2] + parts[3],
        parts[4] + parts[5] + parts[6] + parts[7],
    ]).astype(np.float32)
    if _want_results:
        return out, res
    return out



# revision 7
# speedup vs baseline: 1.3256x; 1.0617x over previous
"""Causal self-attention (B=2, T=2048, C=2048, H=16, Dh=128) on 8 TRN2 NeuronCores.

Sharding: dp=2 over batch x tp=4 over heads (4 heads/core).
  - c_attn column-parallel: each core holds W_attn columns for its 4 heads
    (q, k, v slices), computes qT/kT (head-dim major) and v directly from
    a host-pre-transposed, pre-tiled xT.
  - attention: per-head causal blocks, scoresT in (k, q) orientation; the
    softmax denominator comes free from a ones column appended to v, and
    the exp'd attT blocks feed matmul2 as stationary weights (no
    transposes in the attention inner loop).
  - c_proj row-parallel: each core computes its partial y_heads @ W_proj
    rows; the 4 partials per batch are summed on the host (unshard).

The three stages are software-pipelined per 512-row T-chunk, ordered
q,k-projections -> head-0 scores+exp -> v-projections -> remaining
heads -> deferred c_proj of the previous chunk, so ScalarE exp work
hides under TensorE matmul work of the neighboring sub-stages.

Cold-start path: startup DMAs are issued in exact PE consumption order
(x chunk + k weights interleaved at 2-kt grain, then q, v, proj
weights), and chunk 0 interleaves all four heads' k-projections at the
same 2-kt grain so the PE drains each arriving DMA group slower than
the next one lands (borrowing 2 idle c_proj PSUM banks). Partial
outputs are written in bf16 (the host sums them in f32), with one DMA
per 512-column PSUM group so the drain tail stays short.

All matmuls run in bf16 (inputs pre-cast on host), accumulation fp32.
Host pre-tiles every input so all DMAs are fully contiguous.
"""

import numpy as np
import ml_dtypes

import concourse.bass as bass
import concourse.tile as tile
from concourse import bacc, mybir
from concourse.bass_utils import run_bass_kernel_spmd
from concourse.masks import make_identity, make_upper_triangular

BF16 = mybir.dt.bfloat16
F32 = mybir.dt.float32

B, T, C = 2, 2048, 2048
N_HEAD, D_HEAD = 16, 128
P = 128
KT = C // P          # 16 contraction tiles for qkv projection
NH = 4               # heads per core (tp=4)
TCH = 4              # T chunks of 512 (pipeline granularity)
VW = 129             # v width with appended ones column
WG = 4               # kt-groups per input DMA (finer grain to start PE early)
SCALE = float(1.0 / np.sqrt(D_HEAD))

_CACHE: dict = {}


def _build_program(repeat: int | None = None) -> bacc.Bacc:
    """Build the SPMD program. With `repeat`, the whole body runs inside a
    hardware For_i loop (used only for timing measurements)."""
    import contextlib
    nc = bacc.Bacc("TRN2", target_bir_lowering=False, debug=False)

    # host-pre-tiled layouts (all DMAs contiguous):
    #   xT:  (TCH, P, KT, 512)  xT[tc, p, kt, t'] = x[b][tc*512+t', kt*128+p]
    #   wq/wk/wv: (P, KT, 512)  w[p, kt, m] = W[kt*128+p, m]
    #   wp:  (P, NH, C)         wp[p, h, n] = W_proj[g*512 + h*128+p, n]
    xT_h = nc.dram_tensor("xT", (TCH, P, KT, 512), BF16, kind="ExternalInput")
    wq_h = nc.dram_tensor("wq", (P, KT, NH * P), BF16, kind="ExternalInput")
    wk_h = nc.dram_tensor("wk", (P, KT, NH * P), BF16, kind="ExternalInput")
    wv_h = nc.dram_tensor("wv", (P, KT, NH * P), BF16, kind="ExternalInput")
    wp_h = nc.dram_tensor("wp", (P, NH, C), BF16, kind="ExternalInput")
    out_h = nc.dram_tensor("out", (T, C), BF16, kind="ExternalOutput")

    xT_d = xT_h.ap()
    wq_d, wk_d, wv_d, wp_d = wq_h.ap(), wk_h.ap(), wv_h.ap(), wp_h.ap()
    out_d = out_h.ap().rearrange("(mt p) n -> p mt n", p=P)    # (128, 16, 2048)

    with tile.TileContext(nc) as tc_:
        with (
            tc_.tile_pool(name="consts", bufs=1) as consts,
            tc_.tile_pool(name="persist", bufs=1) as persist,
            tc_.tile_pool(name="wpool", bufs=1) as wpool,
            tc_.tile_pool(name="xpool", bufs=2) as xpool,
            tc_.tile_pool(name="attp", bufs=21) as attp,
            tc_.tile_pool(name="ytp", bufs=2) as ytp,
            tc_.tile_pool(name="ynorm", bufs=3) as ynorm,
            tc_.tile_pool(name="osb", bufs=2) as osb,
            tc_.tile_pool(name="psA", bufs=2, space="PSUM") as psA,
            tc_.tile_pool(name="psS", bufs=2, space="PSUM") as psS,
            tc_.tile_pool(name="psY", bufs=1, space="PSUM") as psY,
            tc_.tile_pool(name="psT", bufs=1, space="PSUM") as psT,
            tc_.tile_pool(name="psO", bufs=2, space="PSUM") as psO,
        ):
            ident = consts.tile([P, P], BF16)
            tri = consts.tile([P, P], BF16)
            make_identity(nc, ident)
            # tri[i, j] = 1 where i <= j (keep k <= q), else 0
            make_upper_triangular(nc, tri, val=1.0, diag=True)

            qT = [persist.tile([P, T], BF16, tag=f"qT{h}", name=f"qT{h}")
                  for h in range(NH)]
            kTt = [persist.tile([P, T], BF16, tag=f"kT{h}", name=f"kT{h}")
                   for h in range(NH)]
            # v_aug for all heads: va_all[:, kk, h, 0:128]=v, [...,128]=1.0
            va_all = persist.tile([P, KT, NH, VW], BF16, tag="va")
            wp_sb = persist.tile([P, NH, C], BF16, tag="wp")

            wq_sb = wpool.tile([P, KT, NH * P], BF16, tag="wq")
            wk_sb = wpool.tile([P, KT, NH * P], BF16, tag="wk")
            wv_sb = wpool.tile([P, KT, NH * P], BF16, tag="wv")

            # first x chunk + k weights first, in the exact order the PE
            # consumes them (gen_qk does the k-projection before q), at
            # 2-kt granularity so the first matmul starts ~2.5us in
            xc0 = xpool.tile([P, KT, 512], BF16, tag="xc", name="xc0")
            # the very first (xc, wk) pair is 1-kt so the first matmul's
            # inputs land ~0.4us sooner; the rest stream at 2-kt grain
            nc.sync.dma_start(xc0[:, 0:1], xT_d[0, :, 0:1])
            nc.sync.dma_start(wk_sb[:, 0:1], wk_d[:, 0:1])
            nc.sync.dma_start(xc0[:, 1:2], xT_d[0, :, 1:2])
            nc.sync.dma_start(wk_sb[:, 1:2], wk_d[:, 1:2])
            FG = 7                      # fine groups for the critical path
            for g in range(FG):
                gs = slice(2 + g * 2, 4 + g * 2)
                nc.sync.dma_start(xc0[:, gs], xT_d[0, :, gs])
                nc.sync.dma_start(wk_sb[:, gs], wk_d[:, gs])
            KG = KT // WG
            for g in range(WG):
                gs = slice(g * KG, (g + 1) * KG)
                nc.sync.dma_start(wq_sb[:, gs], wq_d[:, gs])
            for g in range(WG):
                gs = slice(g * KG, (g + 1) * KG)
                nc.sync.dma_start(wv_sb[:, gs], wv_d[:, gs])
            nc.sync.dma_start(wp_sb[:], wp_d[:])
            # ones column for free softmax denominators
            nc.vector.memset(va_all[:, :, :, D_HEAD:VW], 1.0)

            loop_ctx = tc_.For_i(0, repeat, 1) if repeat else contextlib.nullcontext()

            def emit_proj(tcu, ytc):
                # ---- Stage C(tc): c_proj partial for rows of chunk tc ----
                # emitted after B(tc+1) so it backfills PE during exp stalls
                for j in range(4):
                    mt = tcu * 4 + j
                    o = osb.tile([P, C], BF16, tag="o", name=f"o_{mt}")
                    for nck in range(4):
                        ns = slice(nck * 512, (nck + 1) * 512)
                        po = psO.tile([P, 512], F32, tag="psO", name=f"psO_{mt}_{nck}")
                        for h in range(NH):
                            nc.tensor.matmul(
                                po, ytc[h][:, j * P:(j + 1) * P], wp_sb[:, h, ns],
                                start=(h == 0), stop=(h == NH - 1),
                            )
                        nc.vector.tensor_copy(o[:, ns], po)
                        nc.sync.dma_start(out_d[:, mt, ns], o[:, ns])

            with loop_ctx:
                prev_ytc = None
                xcs = xc0
                for tcu in range(TCH):
                    ts = slice(tcu * 512, (tcu + 1) * 512)
                    xc = xcs
                    if tcu + 1 < TCH:  # prefetch next chunk
                        xcs = xpool.tile([P, KT, 512], BF16, tag="xc",
                                         name=f"xc{tcu + 1}")
                        nc.sync.dma_start(xcs[:], xT_d[tcu + 1])

                    def gen_qk(h2):
                        # q+k projection for head h2, yielding per matmul so
                        # heads 1-3 can fill att@v of the preceding head (the
                        # consumer, attn_scores(h2), runs after mm2(h2-1))
                        hs = slice(h2 * P, (h2 + 1) * P)
                        pk = psA.tile([P, 512], F32, tag="psA",
                                      name=f"pk_{tcu}_{h2}")
                        for kk in range(KT):
                            nc.tensor.matmul(
                                pk, wk_sb[:, kk, hs], xc[:, kk, :],
                                start=(kk == 0), stop=(kk == KT - 1),
                            )
                            yield True
                        nc.vector.tensor_copy(kTt[h2][:, ts], pk)
                        pq = psA.tile([P, 512], F32, tag="psA",
                                      name=f"pq_{tcu}_{h2}")
                        for kk in range(KT):
                            nc.tensor.matmul(
                                pq, wq_sb[:, kk, hs], xc[:, kk, :],
                                start=(kk == 0), stop=(kk == KT - 1),
                            )
                            yield True
                        nc.vector.tensor_copy(qT[h2][:, ts], pq)

                    # ---- Stage A(tc) ----
                    if tcu == 0:
                        # cold start: DMA delivers (xc,wk) 2-kt groups every
                        # ~1.5us; interleave all 4 heads' k-projections at
                        # that grain so the PE consumes each group slower
                        # than it arrives (4 heads x 2 kt x 512 cols =
                        # 1.7us/group), instead of stalling on one head.
                        # psO is free until the first c_proj (chunk 1), so
                        # borrow 2 of its banks for heads 2-3.
                        kps = [psA.tile([P, 512], F32, tag="psA",
                                        name=f"pk0_{h}") for h in (0, 1)]
                        kps += [psO.tile([P, 512], F32, tag="psO",
                                         name=f"pk0_{h}") for h in (2, 3)]
                        for g2 in range(KT // 2):
                            for h in range(NH):
                                for kk in (2 * g2, 2 * g2 + 1):
                                    nc.tensor.matmul(
                                        kps[h], wk_sb[:, kk, h * P:(h + 1) * P],
                                        xc[:, kk, :],
                                        start=(kk == 0), stop=(kk == KT - 1),
                                    )
                        for h in range(NH):
                            nc.vector.tensor_copy(kTt[h][:, ts], kps[h])
                        # q-projection of head 0 only; heads 1-3 stay in the
                        # stage-B weave as mm2 fillers
                        pq0 = psA.tile([P, 512], F32, tag="psA", name="pq0_0")
                        for kk in range(KT):
                            nc.tensor.matmul(
                                pq0, wq_sb[:, kk, 0:P], xc[:, kk, :],
                                start=(kk == 0), stop=(kk == KT - 1),
                            )
                        nc.vector.tensor_copy(qT[0][:, ts], pq0)

                        def gen_q(h2):
                            hs = slice(h2 * P, (h2 + 1) * P)
                            pq = psA.tile([P, 512], F32, tag="psA",
                                          name=f"pq0_{h2}")
                            for kk in range(KT):
                                nc.tensor.matmul(
                                    pq, wq_sb[:, kk, hs], xc[:, kk, :],
                                    start=(kk == 0), stop=(kk == KT - 1),
                                )
                                yield True
                            nc.vector.tensor_copy(qT[h2][:, ts], pq)
                    else:
                        for _ in gen_qk(0):
                            pass

                    qc = tcu

                    def attn_scores(h):
                        # mm1 + exp (+ causal mask) for one head's q chunk
                        att = []
                        for kk in range(4 * qc + 4):
                            ps = psS.tile([P, 512], F32, tag="psS")
                            a = attp.tile([P, 512], BF16, tag="att")
                            if kk < 4 * qc:
                                nc.tensor.matmul(
                                    ps, kTt[h][:, kk * P:(kk + 1) * P], qT[h][:, ts],
                                    start=True, stop=True,
                                )
                                nc.scalar.activation(
                                    a[:], ps[:],
                                    mybir.ActivationFunctionType.Exp, scale=SCALE,
                                )
                            else:
                                off = (kk - 4 * qc) * P
                                nc.tensor.matmul(
                                    ps[:, off:512],
                                    kTt[h][:, kk * P:(kk + 1) * P],
                                    qT[h][:, qc * 512 + off:(qc + 1) * 512],
                                    start=True, stop=True,
                                )
                                nc.scalar.activation(
                                    a[:, off:512], ps[:, off:512],
                                    mybir.ActivationFunctionType.Exp, scale=SCALE,
                                )
                                nc.vector.tensor_tensor(
                                    a[:, off:off + P], a[:, off:off + P], tri,
                                    mybir.AluOpType.mult,
                                )
                            att.append(a)
                        return att

                    # head 0 scores/exp pulled forward: ScalarE works on them
                    # while PE does the v-projections below
                    att0 = attn_scores(0)

                    def gen_vproj(tt):
                        # one v-projection column tile; yields per matmul so it
                        # can be woven between att@v matmuls as 512-col filler
                        pv = psA.tile([P, 512], F32, tag="psA",
                                      name=f"pv_{tcu}_{tt}")
                        for kk in range(KT):
                            nc.tensor.matmul(
                                pv, xc[:, kk, tt * P:(tt + 1) * P],
                                wv_sb[:, kk, :],
                                start=(kk == 0), stop=(kk == KT - 1),
                            )
                            yield True
                        nc.vector.tensor_copy(
                            va_all[:, tcu * 4 + tt, :, 0:D_HEAD],
                            pv.rearrange("p (h d) -> p h d", h=NH),
                        )

                    def gen_cproj(tcv, ytcv):
                        # c_proj of the previous chunk, yielding per matmul
                        for jmt in range(4):
                            mt = tcv * 4 + jmt
                            o = osb.tile([P, C], BF16, tag="o", name=f"o_{mt}")
                            for nck in range(4):
                                ns = slice(nck * 512, (nck + 1) * 512)
                                po = psO.tile([P, 512], F32, tag="psO",
                                              name=f"psO_{mt}_{nck}")
                                for h2 in range(NH):
                                    nc.tensor.matmul(
                                        po, ytcv[h2][:, jmt * P:(jmt + 1) * P],
                                        wp_sb[:, h2, ns],
                                        start=(h2 == 0), stop=(h2 == NH - 1),
                                    )
                                    yield True
                                nc.vector.tensor_copy(o[:, ns], po)
                                nc.sync.dma_start(out_d[:, mt, ns], o[:, ns])

                    # v tiles whose diag-block consumers come too early to be
                    # fed by fillers are emitted up front; the rest (plus the
                    # deferred c_proj) interleave 1:1 with the narrow att@v
                    # matmuls, hiding their ~139ns issue floor under 512-col
                    # streams (measured: a 512+129 pair costs ~305ns vs 368).
                    import itertools
                    if tcu == 0:
                        up_tt = 4
                    elif tcu == 1:
                        up_tt = 3
                    else:
                        up_tt = 2
                    for tt in range(up_tt):
                        for _ in gen_vproj(tt):
                            pass
                    # per-head primary fillers: mm2(h) hides vproj tails /
                    # the NEXT head's q,k projection; shared tail = c_proj
                    if tcu == 0:
                        primary = {
                            0: gen_q(1),
                            1: gen_q(2),
                            2: gen_q(3),
                            3: iter(()),
                        }
                    else:
                        primary = {
                            0: itertools.chain(
                                *[gen_vproj(tt) for tt in range(up_tt, 4)],
                                gen_qk(1)),
                            1: gen_qk(2),
                            2: gen_qk(3),
                            3: iter(()),
                        }
                    tail = (gen_cproj(tcu - 1, prev_ytc)
                            if prev_ytc is not None else iter(()))

                    # ---- Stage B(qc=tc): remaining heads + weighted sums ----
                    ytc = []
                    for h in range(NH):
                        att = att0 if h == 0 else attn_scores(h)

                        yt = ytp.tile([P, 512], BF16, tag=f"yT{h}", name=f"yT{h}_{qc}")
                        ytc.append(yt)
                        pri = primary[h]
                        for j in range(4):
                            qt = qc * 4 + j
                            py = psY.tile([P, VW], F32, tag="psY")
                            for kk in range(qt + 1):
                                nc.tensor.matmul(
                                    py, att[kk][:, j * P:(j + 1) * P],
                                    va_all[:, kk, h, :],
                                    start=(kk == 0), stop=(kk == qt),
                                )
                                if next(pri, None) is None:
                                    next(tail, None)
                            r = ynorm.tile([P, 1], F32, tag="r")
                            nc.vector.reciprocal(r, py[:, D_HEAD:VW])
                            y = ynorm.tile([P, P], BF16, tag="y")
                            nc.vector.tensor_scalar_mul(y, py[:, 0:D_HEAD], r)
                            pt = psT.tile([P, P], BF16, tag="psT")
                            nc.tensor.transpose(pt, y, ident)
                            nc.vector.tensor_copy(yt[:, j * P:(j + 1) * P], pt)
                        # next head's q,k must be complete before its scores
                        for _ in pri:
                            pass

                    for _ in tail:
                        pass
                    prev_ytc = ytc

                emit_proj(TCH - 1, prev_ytc)

    nc.compile()
    return nc


def _get_program() -> bacc.Bacc:
    if "nc" not in _CACHE:
        _CACHE["nc"] = _build_program()
    return _CACHE["nc"]


def _make_in_maps(x, W_attn, W_proj):
    bf = ml_dtypes.bfloat16
    x = np.asarray(x, dtype=np.float32)
    W_attn = np.asarray(W_attn, dtype=np.float32)
    W_proj = np.asarray(W_proj, dtype=np.float32)

    # xT[tc, p, kt, t'] = x[b][tc*512+t', kt*128+p]
    xT_b = []
    for b in range(B):
        xt = x[b].T.reshape(KT, P, TCH, 512).transpose(2, 1, 0, 3)
        xT_b.append(np.ascontiguousarray(xt).astype(bf))

    def _tile_w(w):  # (C, 512) -> (P, KT, 512)
        return np.ascontiguousarray(
            w.reshape(KT, P, NH * P).transpose(1, 0, 2)).astype(bf)

    GW = NH * D_HEAD  # 512 columns per tp group
    in_maps = []
    for core in range(8):
        b, g = divmod(core, 4)
        wp = W_proj[g * GW:(g + 1) * GW, :].reshape(NH, P, C).transpose(1, 0, 2)
        in_maps.append({
            "xT": xT_b[b],
            "wq": _tile_w(W_attn[:, g * GW:(g + 1) * GW]),
            "wk": _tile_w(W_attn[:, C + g * GW: C + (g + 1) * GW]),
            "wv": _tile_w(W_attn[:, 2 * C + g * GW: 2 * C + (g + 1) * GW]),
            "wp": np.ascontiguousarray(wp).astype(bf),
        })
    return in_maps


def kernel(x, W_attn, W_proj, _want_results=False, _trace=False):
    nc = _get_program()
    in_maps = _make_in_maps(x, W_attn, W_proj)
    res = run_bass_kernel_spmd(
        nc, in_maps, core_ids=list(range(8)), trace=_trace,
    )
    parts = [np.asarray(res.results[i]["out"], dtype=np.float32) for i in range(8)]
    out = np.stack([
        parts[0] + parts[1] + parts[2] + parts[3],
        parts[4] + parts[5] + parts[6] + parts[7],
    ]).astype(np.float32)
    if _want_results:
        return out, res
    return out

